# revision 33
# baseline (speedup 1.0000x reference)
"""Multi-head causal attention (B=4, S=2048, D=1024, H=16) on 8 TRN2 NeuronCores.

Sharding: data-parallel over batch (4) x tensor-parallel over heads (2 groups
of 8 heads) = 8 cores. Each core computes, for its (batch, head-group):
  Q^T/K^T = (x @ Wq/Wk)^T   [dc, S]   (dc = 512 head-group dims)
  V       = x @ Wv          [S, dc]
  per head h, per 512-wide query block iB (flash-style, scores transposed):
    E[j, i]      = exp(scoresT / 8) with causal mask (j <= i), j tiled by 128
    attnoutT|r   = [V_h | ones].T @ E   -> [65, i]  (row 64 = softmax denom r)
    anorm        = attnoutT * (1/r)      (broadcast over d)
  out_partial[i, :] += anorm_h.T @ (W_o[:, cols].T)   accumulated over heads
Host sums the two head-group partials per batch (the W_o row-shard
all-reduce from the sharding hint, done on host during unshard).

v2: all matmul operands bf16 (tolerance is 2e-2; bf16 keeps us ~5e-3).
QKV projection and attention are emitted interleaved (per-chunk tiles give
tile-granular deps) so the Tile scheduler uses projection matmuls of block
sb+1 as PE filler while attention block iB=sb waits on the scalar-engine
exp — this keeps the PE HAM-warm at 2.4 GHz. Diagonal key-blocks trim the
score/AV matmuls and masks to the causally-valid query range.
"""

import sys

if "/opt/trn_rl_repo" not in sys.path:
    sys.path.insert(0, "/opt/trn_rl_repo")

import numpy as np
import ml_dtypes

import concourse.bacc as bacc
import concourse.mybir as mybir
import concourse.tile as tile
from concourse.bass import ts
from concourse.bass_utils import run_bass_kernel_spmd

F32 = mybir.dt.float32
BF16 = mybir.dt.bfloat16
AF = mybir.ActivationFunctionType

B, S, D, H = 4, 2048, 1024, 16
HD = D // H           # 64
NCORES = 8
HG = 8                # heads per core
DC = HG * HD          # 512 feature cols per core
SB = 512              # s-block
NSB = S // SB         # 4
KC = D // 128         # 8 k-chunks
NIB = S // SB         # 4 query blocks
SCALE = 1.0 / np.sqrt(HD)

_cached_nc = None


def _build():
    nc = bacc.Bacc("TRN2", target_bir_lowering=False, debug=False)

    xt_d = nc.dram_tensor("xt", [D, S], BF16, kind="ExternalInput")      # x[b].T
    wq_d = nc.dram_tensor("wq", [D, DC], BF16, kind="ExternalInput")
    wk_d = nc.dram_tensor("wk", [D, DC], BF16, kind="ExternalInput")
    wv_d = nc.dram_tensor("wv", [D, DC], BF16, kind="ExternalInput")
    wot_d = nc.dram_tensor("wot", [DC, D], BF16, kind="ExternalInput")   # W_o[:, cols].T
    out_d = nc.dram_tensor("out", [S, D], BF16, kind="ExternalOutput")

    with tile.TileContext(nc) as tc:
        with (
            tc.tile_pool(name="xp", bufs=32) as x_pool,
            tc.tile_pool(name="wp", bufs=8) as w_pool,
            tc.tile_pool(name="qkp", bufs=16) as qk_pool,
            tc.tile_pool(name="vp", bufs=16) as v_pool,
            tc.tile_pool(name="wotp", bufs=4) as wot_pool,
            tc.tile_pool(name="ep", bufs=4) as e_pool,
            tc.tile_pool(name="t65p", bufs=4) as t65_pool,
            tc.tile_pool(name="rp", bufs=4) as r_pool,
            tc.tile_pool(name="bcp", bufs=3) as bc_pool,
            tc.tile_pool(name="anp", bufs=9) as an_pool,
            tc.tile_pool(name="op", bufs=3) as o_pool,
            tc.tile_pool(name="ps_qkv", bufs=2, space="PSUM") as ps_qkv,
            tc.tile_pool(name="ps_mm", bufs=2, space="PSUM") as ps_mm,
            tc.tile_pool(name="ps_acc", bufs=2, space="PSUM") as ps_acc,
        ):
            # persistent chunked tiles (tile-granular deps let the scheduler
            # interleave QKV of block sb+1 with attention of block sb)
            xt_c = [[x_pool.tile([128, SB], BF16, tag="xt", name=f"xt{sb}_{kc}")
                     for kc in range(KC)] for sb in range(NSB)]
            wq_t = [w_pool.tile([128, DC], BF16, tag="wq", name=f"wq{kc}")
                    for kc in range(KC)]
            wk_t = [w_pool.tile([128, DC], BF16, tag="wk", name=f"wk{kc}")
                    for kc in range(KC)]
            wv_t = [w_pool.tile([128, DC], BF16, tag="wv", name=f"wv{kc}")
                    for kc in range(KC)]
            qt_c = [[qk_pool.tile([128, SB], BF16, tag="qt", name=f"qt{m}_{sb}")
                     for sb in range(NSB)] for m in range(4)]
            kt_c = [[qk_pool.tile([128, SB], BF16, tag="kt", name=f"kt{m}_{sb}")
                     for sb in range(NSB)] for m in range(4)]
            vt = [v_pool.tile([128, HG, HD + 1], BF16, tag="vt", name=f"vt{j}")
                  for j in range(16)]
            wot = [wot_pool.tile([128, D], BF16, tag="wot", name=f"wot{t}")
                   for t in range(4)]
            # junk bf16 weights: dependency-free LDWEIGHTS emitted between
            # real matmuls keep the PE activity monitor from down-clocking
            junk16 = wot_pool.tile([128, 128], BF16, tag="junk", bufs=1)
            nc.vector.memset(junk16, 0.0)
            for j in range(16):
                nc.vector.memset(vt[j][:, :, HD:HD + 1], 1.0)

            def emit_in_dma(sb):
                # first block: xt rides the SP HWDGE queue while the weights
                # ride the ACT HWDGE queue (idle at startup) — both DMA rings
                # ramp in parallel so the first matmul group unblocks early.
                # Later xt blocks alternate queues so filler QKV data lands
                # before the attention block that needs it as PE filler.
                for kc in range(KC):
                    eng = nc.sync if (sb == 0 or kc % 2 == 0) else nc.scalar
                    eng.dma_start(out=xt_c[sb][kc],
                                  in_=xt_d[ts(kc, 128), ts(sb, SB)])
                    if sb == 0:
                        nc.scalar.dma_start(out=wq_t[kc], in_=wq_d[ts(kc, 128), :])
                if sb == 0:
                    for kc in range(KC):
                        nc.scalar.dma_start(out=wk_t[kc], in_=wk_d[ts(kc, 128), :])
                    for kc in range(KC):
                        nc.scalar.dma_start(out=wv_t[kc], in_=wv_d[ts(kc, 128), :])
                    for t in range(4):
                        nc.scalar.dma_start(out=wot[t], in_=wot_d[ts(t, 128), :])

            def emit_qkv(sb):
                for m in range(4):
                    for w_t, dst in ((wq_t, qt_c), (wk_t, kt_c)):
                        ps = ps_qkv.tile([128, SB], F32, tag="qkv")
                        for kc in range(KC):
                            nc.tensor.matmul(
                                ps, w_t[kc][:, ts(m, 128)], xt_c[sb][kc],
                                start=(kc == 0), stop=(kc == KC - 1),
                            )
                        nc.vector.tensor_copy(dst[m][sb], ps)
                for sc in range(4):
                    ps = ps_qkv.tile([128, DC], F32, tag="qkv")
                    for kc in range(KC):
                        nc.tensor.matmul(
                            ps, xt_c[sb][kc][:, ts(sc, 128)], wv_t[kc],
                            start=(kc == 0), stop=(kc == KC - 1),
                        )
                    nc.vector.tensor_copy(
                        vt[4 * sb + sc][:, :, 0:HD],
                        ps.rearrange("p (h d) -> p h d", h=HG),
                    )

            def emit_att(iB):
                nu = 2 * (iB + 1)
                anorms = [None] * 4
                for hp in range(4):
                    h0, h1 = 2 * hp, 2 * hp + 1
                    m = hp
                    accs = {h: ps_acc.tile([128, SB], F32, tag="acc",
                                           name=f"acc_{iB}_{h}")
                            for h in (h0, h1)}
                    for u in range(nu):
                        jj0, jj1 = 2 * u, 2 * u + 1
                        pss = {}
                        for h in (h0, h1):
                            rb = (h % 2) * 64
                            ps2 = ps_mm.tile([128, 1024], F32, tag="mm",
                                             name=f"ps_{iB}_{u}_{h}")
                            pss[h] = ps2
                            for q, jj in ((0, jj0), (1, jj1)):
                                t = jj - 4 * iB
                                a = 128 * t if t > 0 else 0
                                nc.tensor.matmul(
                                    ps2[:, q * SB + a:(q + 1) * SB],
                                    kt_c[m][jj // 4][rb:rb + 64, ts(jj % 4, 128)],
                                    qt_c[m][iB][rb:rb + 64, a:SB],
                                    start=True, stop=True,
                                )
                        nc.tensor.ldweights(junk16)
                        t0 = jj0 - 4 * iB
                        a0 = 128 * t0 if t0 > 0 else 0
                        es = {}
                        for h in (h0, h1):
                            e_t = e_pool.tile([128, 1024], BF16, tag="e",
                                              name=f"e_{iB}_{u}_{h}")
                            es[h] = e_t
                            nc.scalar.activation(e_t[:, a0:1024], pss[h][:, a0:1024],
                                                 AF.Exp, scale=float(SCALE))
                        for h in (h0, h1):
                            for q, jj in ((0, jj0), (1, jj1)):
                                t = jj - 4 * iB
                                if t >= 0:
                                    strip = es[h][:, q * SB + 128 * t:
                                                  q * SB + 128 * t + 128]
                                    nc.gpsimd.affine_select(
                                        out=strip, in_=strip,
                                        pattern=[[1, 128]],
                                        compare_op=mybir.AluOpType.is_ge,
                                        fill=0.0, base=0,
                                        channel_multiplier=-1,
                                    )
                        for h in (h0, h1):
                            for q, jj in ((0, jj0), (1, jj1)):
                                t = jj - 4 * iB
                                a = 128 * t if t > 0 else 0
                                nc.tensor.matmul(
                                    accs[h][0:HD + 1, a:SB],
                                    vt[jj][:, h, :],
                                    es[h][:, q * SB + a:(q + 1) * SB],
                                    start=(u == 0 and q == 0),
                                    stop=(u == nu - 1 and q == 1),
                                )
                        nc.tensor.ldweights(junk16)
                    an_pair = an_pool.tile([128, SB], BF16, tag="an",
                                           name=f"an_{iB}_{hp}")
                    anorms[hp] = an_pair
                    for h in (h0, h1):
                        t65 = t65_pool.tile([HD + 1, SB], F32, tag="t65",
                                            name=f"t65_{iB}_{h}")
                        nc.vector.tensor_copy(t65, accs[h][0:HD + 1, :])
                        r_t = r_pool.tile([1, SB], F32, tag="r", name=f"r_{iB}_{h}")
                        nc.sync.dma_start(out=r_t, in_=t65[HD:HD + 1, :])
                        rec_t = r_pool.tile([1, SB], F32, tag="rec",
                                            name=f"rec_{iB}_{h}")
                        nc.vector.reciprocal_approx_fast(out=rec_t, in_=r_t)
                        bc_t = bc_pool.tile([HD, SB], F32, tag="bc",
                                            name=f"bc_{iB}_{h}")
                        nc.gpsimd.partition_broadcast(bc_t, rec_t)
                        if h == h0:
                            nc.vector.tensor_mul(an_pair[0:HD, :],
                                                 t65[0:HD, :], bc_t)
                        else:
                            an_odd = an_pool.tile([HD, SB], BF16, tag="anodd",
                                                  name=f"anodd_{iB}_{h}")
                            nc.vector.tensor_mul(an_odd, t65[0:HD, :], bc_t)
                            # stage odd head to partitions 64..127 (DMA can
                            # move across partitions; DVE cannot)
                            nc.sync.dma_start(out=an_pair[HD:128, :], in_=an_odd)

                return anorms

            def emit_proj(iB, anorms):
                # output projection for query block iB, summed over heads.
                # Emission is deferred one block so these matmuls fill PE
                # stalls of the (scalar-bound) next attention block. po rides
                # the ps_acc ring; QKV filler keeps its own pool.
                for ic in range(4):
                    o_t = o_pool.tile([128, D], BF16, tag="o")
                    for dh in range(2):
                        po = ps_acc.tile([128, 512], F32, tag="acc",
                                         name=f"po_{iB}_{ic}_{dh}")
                        for hp2 in range(4):
                            nc.tensor.matmul(
                                po,
                                anorms[hp2][:, ts(ic, 128)],
                                wot[hp2][:, ts(dh, 512)],
                                start=(hp2 == 0), stop=(hp2 == 3),
                            )
                        nc.tensor.ldweights(junk16)
                        nc.vector.tensor_copy(o_t[:, ts(dh, 512)], po)
                    # final block: spread the output over both HWDGE queues
                    # to shrink the drain tail
                    eng = nc.scalar if (iB == 3 and ic % 2 == 1) else nc.sync
                    eng.dma_start(
                        out=out_d[iB * SB + ic * 128:iB * SB + (ic + 1) * 128, :],
                        in_=o_t,
                    )

            emit_in_dma(0)
            emit_qkv(0)
            ans = {}
            emit_in_dma(1)
            ans[0] = emit_att(0)
            emit_qkv(1)
            emit_in_dma(2)
            ans[1] = emit_att(1)
            emit_proj(0, ans[0])
            emit_qkv(2)
            emit_in_dma(3)
            ans[2] = emit_att(2)
            emit_proj(1, ans[1])
            emit_qkv(3)
            ans[3] = emit_att(3)
            emit_proj(2, ans[2])
            emit_proj(3, ans[3])

    nc.compile()
    return nc


def make_in_maps(x, W_q, W_k, W_v, W_o):
    bf16 = ml_dtypes.bfloat16
    x = np.asarray(x, dtype=np.float32)
    in_maps = []
    for c in range(NCORES):
        b, g = c // 2, c % 2
        cols = slice(g * DC, (g + 1) * DC)
        in_maps.append({
            "xt": np.ascontiguousarray(x[b].T.astype(bf16)),
            "wq": np.ascontiguousarray(np.asarray(W_q)[:, cols].astype(bf16)),
            "wk": np.ascontiguousarray(np.asarray(W_k)[:, cols].astype(bf16)),
            "wv": np.ascontiguousarray(np.asarray(W_v)[:, cols].astype(bf16)),
            "wot": np.ascontiguousarray(np.asarray(W_o)[:, cols].T.astype(bf16)),
        })
    return in_maps


def kernel(x, W_q, W_k, W_v, W_o):
    global _cached_nc
    if _cached_nc is None:
        _cached_nc = _build()
    nc = _cached_nc

    in_maps = make_in_maps(x, W_q, W_k, W_v, W_o)
    res = run_bass_kernel_spmd(nc, in_maps, list(range(NCORES))).results
    out = np.empty((B, S, D), np.float32)
    for b in range(B):
        out[b] = (res[2 * b]["out"].astype(np.float32)
                  + res[2 * b + 1]["out"].astype(np.float32))
    return out
